# revision 3
# baseline (speedup 1.0000x reference)
"""CIDER criterion (DisLoss + CompLoss) on 8 Trainium2 NeuronCores.

Strategy
--------
The reference does (1) an order-dependent EMA prototype update scan over the
batch, (2) a prototype-prototype similarity loss, (3) a feature-prototype
cross-entropy loss.  Updates for different classes are independent, so the
scan is reorganized into per-class chains (max chain length L ~= 19 for
B=8192, C=1000) and the 1000 chains are sharded over the 8 cores (125
classes/core, one 128-partition tile each).

Stage A (per core): run L masked scan steps
    q = p + f_t;  p = q / max(||q||, eps)
on its 125 classes; also accumulates per-class feature sums to produce
sum_i <f_i, p_{label_i}> (the CompLoss positive term) without any gather,
plus per-class ||p||^2 (the DisLoss diagonal term).

Stage B (per core): comp logits for its 1024 batch rows (features.T chunk @
proto.T, contraction over D on the PE), row logsumexp with the max folded
into the ACT Exp pass; dis logits for its 125 prototype rows vs all 1000,
masked row sums via (full sum - diag).  Each core returns two partial
scalars; the host combines ~10 scalar flops at the end.

Host work is limited to index prep (argsort of labels), layout transforms
(transpose / slicing / padding), and the final 8-way scalar combine.
"""

import numpy as np

# ---- problem constants (hardcoded per the harness contract) ----
B, C, D = 8192, 1000, 512
NCORES = 8
CPC = C // NCORES  # 125 classes per core
BPC = B // NCORES  # 1024 batch rows per core
P = 128
NHALF = 500  # class-column chunk (PSUM bank = 512 f32 max)
KT = D // P  # 4 contraction chunks
MT = BPC // P  # 8 batch chunks per core

_CACHE = {}




def _build_stage_a(L):
    """Per-class EMA scan over L steps for 125 classes (rows on partitions)."""
    from contextlib import ExitStack

    import concourse.bacc as bacc
    import concourse.tile as tile
    from concourse import mybir

    f32 = mybir.dt.float32
    OP = mybir.AluOpType
    AF = mybir.ActivationFunctionType

    nc = bacc.Bacc(None)
    sf = nc.dram_tensor("scan_feats", [L, P, D], f32, kind="ExternalInput")
    p0 = nc.dram_tensor("proto_init", [P, D], f32, kind="ExternalInput")
    proto_out = nc.dram_tensor("proto_out", [P, D], f32, kind="ExternalOutput")
    rowsq_out = nc.dram_tensor("rowsq_out", [P, 1], f32, kind="ExternalOutput")
    possum_out = nc.dram_tensor("possum_out", [1, 1], f32, kind="ExternalOutput")

    with tile.TileContext(nc) as tc, ExitStack() as ctx:
        persist = ctx.enter_context(tc.tile_pool(name="persist", bufs=1))
        fpool = ctx.enter_context(tc.tile_pool(name="fpool", bufs=3))
        qpool = ctx.enter_context(tc.tile_pool(name="qpool", bufs=2))
        scrp = ctx.enter_context(tc.tile_pool(name="scrp", bufs=2))
        small = ctx.enter_context(tc.tile_pool(name="small", bufs=4))
        psum = ctx.enter_context(tc.tile_pool(name="psum", bufs=1, space="PSUM"))

        p = persist.tile([P, D], f32)
        nc.sync.dma_start(out=p[:], in_=p0[:, :])
        cs = persist.tile([P, D], f32)
        nc.vector.memset(cs[:], 0.0)

        for t in range(L):
            f = fpool.tile([P, D], f32, tag="f")
            nc.sync.dma_start(out=f[:], in_=sf[t])
            q = qpool.tile([P, D], f32, tag="q")
            nc.vector.tensor_add(out=q[:], in0=p[:], in1=f[:])
            scr = scrp.tile([P, D], f32, tag="scr")
            ssq = small.tile([P, 1], f32, tag="ssq")
            nc.scalar.activation(
                out=scr[:], in_=q[:], func=AF.Square, accum_out=ssq[:]
            )
            n = small.tile([P, 1], f32, tag="n")
            nc.scalar.sqrt(n[:], ssq[:])
            nm = small.tile([P, 1], f32, tag="nm")
            nc.vector.tensor_scalar_max(out=nm[:], in0=n[:], scalar1=1e-12)
            r = small.tile([P, 1], f32, tag="r")
            nc.vector.reciprocal(out=r[:], in_=nm[:])
            nc.vector.tensor_scalar_mul(out=p[:], in0=q[:], scalar1=r[:])
            nc.gpsimd.tensor_add(out=cs[:], in0=cs[:], in1=f[:])

        dotv = small.tile([P, 1], f32, tag="dotv")
        scr = scrp.tile([P, D], f32, tag="scr")
        nc.vector.tensor_mul(out=scr[:], in0=cs[:], in1=p[:])
        nc.vector.reduce_sum(out=dotv[:], in_=scr[:], axis=mybir.AxisListType.X)
        rsq = small.tile([P, 1], f32, tag="rsq")
        scr2 = scrp.tile([P, D], f32, tag="scr")
        nc.scalar.activation(
            out=scr2[:], in_=p[:], func=AF.Square, accum_out=rsq[:]
        )
        ones = persist.tile([P, 1], f32)
        nc.vector.memset(ones[:], 1.0)
        ps = psum.tile([1, 1], f32)
        nc.tensor.matmul(ps[:], lhsT=ones[:], rhs=dotv[:], start=True, stop=True)
        poss_sb = small.tile([1, 1], f32, tag="poss")
        nc.vector.tensor_copy(out=poss_sb[:], in_=ps[:])

        nc.sync.dma_start(out=proto_out[:, :], in_=p[:])
        nc.sync.dma_start(out=rowsq_out[:, :], in_=rsq[:])
        nc.sync.dma_start(out=possum_out[:, :], in_=poss_sb[:])
    nc.finalize()
    return nc


def _build_stage_b():
    """Comp logits + row logsumexp for 1024 batch rows; dis logits + masked
    row sums for 125 prototype rows.  Two partial scalars out."""
    from contextlib import ExitStack

    import concourse.bacc as bacc
    import concourse.tile as tile
    from concourse import mybir

    f32 = mybir.dt.float32
    OP = mybir.AluOpType
    AF = mybir.ActivationFunctionType

    nc = bacc.Bacc(None)
    featT = nc.dram_tensor("featT", [D, BPC], f32, kind="ExternalInput")
    protoT = nc.dram_tensor("protoT", [D, C], f32, kind="ExternalInput")
    protoT_own = nc.dram_tensor("protoT_own", [D, CPC], f32, kind="ExternalInput")
    rowsq_own = nc.dram_tensor("rowsq_own", [CPC, 1], f32, kind="ExternalInput")
    comp_out = nc.dram_tensor("comp_out", [1, 1], f32, kind="ExternalOutput")
    dis_out = nc.dram_tensor("dis_out", [1, 1], f32, kind="ExternalOutput")

    with tile.TileContext(nc) as tc, ExitStack() as ctx:
        singles = ctx.enter_context(tc.tile_pool(name="singles", bufs=1))
        scrp = ctx.enter_context(tc.tile_pool(name="scrp", bufs=2))
        small = ctx.enter_context(tc.tile_pool(name="small", bufs=4))
        pp = ctx.enter_context(tc.tile_pool(name="pp", bufs=4, space="PSUM"))
        pred = ctx.enter_context(tc.tile_pool(name="pred", bufs=2, space="PSUM"))

        pt = []
        for k in range(KT):
            t_ = singles.tile([P, C], f32, tag=f"pt{k}")
            nc.sync.dma_start(out=t_[:], in_=protoT[k * P : (k + 1) * P, :])
            pt.append(t_)
        po = []
        for k in range(KT):
            t_ = singles.tile([P, CPC], f32, tag=f"po{k}")
            nc.sync.dma_start(out=t_[:], in_=protoT_own[k * P : (k + 1) * P, :])
            po.append(t_)
        ft = []
        for k in range(KT):
            t_ = singles.tile([P, BPC], f32, tag=f"ft{k}")
            nc.sync.dma_start(out=t_[:], in_=featT[k * P : (k + 1) * P, :])
            ft.append(t_)
        rsq = singles.tile([CPC, 1], f32, tag="rsq")
        nc.sync.dma_start(out=rsq[:], in_=rowsq_own[:, :])
        ones = singles.tile([P, 1], f32, tag="ones")
        nc.vector.memset(ones[:], 1.0)

        # ---------------- DisLoss rows ----------------
        ses_d = small.tile([CPC, 2], f32, tag="ses_d")
        for nk in range(2):
            pd = pp.tile([CPC, NHALF], f32, tag="pp")
            for k in range(KT):
                nc.tensor.matmul(
                    pd[:], lhsT=po[k][:], rhs=pt[k][:, nk * NHALF : (nk + 1) * NHALF],
                    start=(k == 0), stop=(k == KT - 1),
                )
            e = scrp.tile([P, NHALF], f32, tag="escr")
            nc.scalar.activation(
                out=e[:CPC, :], in_=pd[:], func=AF.Exp, scale=10.0,
                accum_out=ses_d[:, nk : nk + 1],
            )
        rowsum = small.tile([CPC, 1], f32, tag="rowsum")
        nc.vector.reduce_sum(out=rowsum[:], in_=ses_d[:], axis=mybir.AxisListType.X)
        diag = small.tile([CPC, 1], f32, tag="diag")
        nc.scalar.activation(out=diag[:], in_=rsq[:], func=AF.Exp, scale=10.0)
        lnfull = singles.tile([P, 1], f32, tag="lnfull")
        nc.vector.memset(lnfull[:], 0.0)
        masked = small.tile([CPC, 1], f32, tag="masked")
        nc.vector.tensor_sub(out=masked[:], in0=rowsum[:], in1=diag[:])
        nc.scalar.activation(out=lnfull[:CPC, :], in_=masked[:], func=AF.Ln)
        psd = pred.tile([1, 1], f32, tag="pred")
        nc.tensor.matmul(psd[:], lhsT=ones[:], rhs=lnfull[:], start=True, stop=True)
        dis_sb = small.tile([1, 1], f32, tag="dis_sb")
        nc.vector.tensor_copy(out=dis_sb[:], in_=psd[:])
        nc.sync.dma_start(out=dis_out[:, :], in_=dis_sb[:])

        # ---------------- CompLoss rows ----------------
        mx_all = singles.tile([P, MT], f32, tag="mx_all")
        ses_all = singles.tile([P, MT], f32, tag="ses_all")
        for m in range(MT):
            pc = []
            for nk in range(2):
                ptile = pp.tile([P, NHALF], f32, tag="pp")
                for k in range(KT):
                    nc.tensor.matmul(
                        ptile[:],
                        lhsT=ft[k][:, m * P : (m + 1) * P],
                        rhs=pt[k][:, nk * NHALF : (nk + 1) * NHALF],
                        start=(k == 0), stop=(k == KT - 1),
                    )
                pc.append(ptile)
            m0 = small.tile([P, 1], f32, tag="m0")
            nc.vector.reduce_max(out=m0[:], in_=pc[0][:], axis=mybir.AxisListType.X)
            m1 = small.tile([P, 1], f32, tag="m1")
            nc.vector.reduce_max(out=m1[:], in_=pc[1][:], axis=mybir.AxisListType.X)
            nc.vector.tensor_max(out=mx_all[:, m : m + 1], in0=m0[:], in1=m1[:])
            negb = small.tile([P, 1], f32, tag="negb")
            nc.vector.tensor_scalar_mul(
                out=negb[:], in0=mx_all[:, m : m + 1], scalar1=-10.0
            )
            ses01 = small.tile([P, 2], f32, tag="ses01")
            for nk in range(2):
                e = scrp.tile([P, NHALF], f32, tag="escr")
                nc.scalar.activation(
                    out=e[:], in_=pc[nk][:], func=AF.Exp, bias=negb[:], scale=10.0,
                    accum_out=ses01[:, nk : nk + 1],
                )
            nc.vector.reduce_sum(
                out=ses_all[:, m : m + 1], in_=ses01[:], axis=mybir.AxisListType.X
            )
        ln_all = singles.tile([P, MT], f32, tag="ln_all")
        nc.scalar.activation(out=ln_all[:], in_=ses_all[:], func=AF.Ln)
        term = singles.tile([P, MT], f32, tag="term")
        nc.vector.tensor_scalar_mul(out=term[:], in0=mx_all[:], scalar1=10.0)
        nc.vector.tensor_add(out=term[:], in0=term[:], in1=ln_all[:])
        tvec = small.tile([P, 1], f32, tag="tvec")
        nc.vector.reduce_sum(out=tvec[:], in_=term[:], axis=mybir.AxisListType.X)
        psc = pred.tile([1, 1], f32, tag="pred")
        nc.tensor.matmul(psc[:], lhsT=ones[:], rhs=tvec[:], start=True, stop=True)
        comp_sb = small.tile([1, 1], f32, tag="comp_sb")
        nc.vector.tensor_copy(out=comp_sb[:], in_=psc[:])
        nc.sync.dma_start(out=comp_out[:, :], in_=comp_sb[:])
    nc.finalize()
    return nc


def _get_stage_a(L):
    key = ("A", L)
    if key not in _CACHE:
        _CACHE[key] = _build_stage_a(L)
    return _CACHE[key]


def _get_stage_b():
    if "B" not in _CACHE:
        _CACHE["B"] = _build_stage_b()
    return _CACHE["B"]


def kernel(features, prototypes, labels):
    from concourse.bass_utils import run_bass_kernel_spmd

    f32 = np.float32
    features = np.ascontiguousarray(features, dtype=f32)
    prototypes = np.ascontiguousarray(prototypes, dtype=f32)
    labels = np.asarray(labels)

    # ---- host index prep: per-class ordered sample lists ----
    order = np.argsort(labels, kind="stable")
    counts = np.bincount(labels, minlength=C)
    L = max(int(counts.max()), 1)
    starts = np.concatenate([[0], np.cumsum(counts)])
    sorted_feats = features[order]
    lab_sorted = labels[order]
    slot = np.arange(B) - starts[lab_sorted]
    core_of = lab_sorted // CPC
    row_in_core = lab_sorted % CPC

    sf_all = np.zeros((NCORES, L, P, D), f32)
    sf_all[core_of, slot, row_in_core] = sorted_feats
    pi_all = np.zeros((NCORES, P, D), f32)
    for c in range(NCORES):
        pi_all[c, :CPC] = prototypes[c * CPC : (c + 1) * CPC]

    # ---- stage A on device ----
    ncA = _get_stage_a(L)
    in_maps = [
        {"scan_feats": sf_all[c], "proto_init": pi_all[c]} for c in range(NCORES)
    ]
    resA = run_bass_kernel_spmd(ncA, in_maps, list(range(NCORES))).results

    proto = np.concatenate([resA[c]["proto_out"][:CPC] for c in range(NCORES)])
    rowsq = np.concatenate([resA[c]["rowsq_out"][:CPC, 0] for c in range(NCORES)])
    possum = np.sum(
        np.array([resA[c]["possum_out"][0, 0] for c in range(NCORES)], f32), dtype=f32
    )

    # ---- stage B on device ----
    protoT = np.ascontiguousarray(proto.T)
    featT = np.ascontiguousarray(features.T)
    ncB = _get_stage_b()
    in_maps = [
        {
            "featT": np.ascontiguousarray(featT[:, c * BPC : (c + 1) * BPC]),
            "protoT": protoT,
            "protoT_own": np.ascontiguousarray(protoT[:, c * CPC : (c + 1) * CPC]),
            "rowsq_own": np.ascontiguousarray(
                rowsq[c * CPC : (c + 1) * CPC].reshape(CPC, 1)
            ),
        }
        for c in range(NCORES)
    ]
    resB = run_bass_kernel_spmd(ncB, in_maps, list(range(NCORES))).results

    comp_total = np.sum(
        np.array([resB[c]["comp_out"][0, 0] for c in range(NCORES)], f32), dtype=f32
    )
    dis_total = np.sum(
        np.array([resB[c]["dis_out"][0, 0] for c in range(NCORES)], f32), dtype=f32
    )

    # ---- final scalar combine (the unshard step) ----
    mean_log_prob_pos = (f32(10.0) * possum - comp_total) / f32(B)
    loss_comp = -mean_log_prob_pos
    loss_dis = dis_total / f32(C) - np.log(f32(C - 1))
    return np.array(loss_comp + loss_dis, dtype=f32)
